# revision 25
# baseline (speedup 1.0000x reference)
import os
import sys
import types
from contextlib import ExitStack

sys.path.insert(0, "/opt/trn_rl_repo")

import numpy as np
from ml_dtypes import bfloat16 as np_bf16

import concourse.bacc as bacc
import concourse.tile as tile
import concourse.mybir as mybir
from concourse import bass_utils, masks
from concourse.bass_utils import run_bass_kernel_spmd

NCORES = 8
B, N, HX, HS = 32, 4096, 128, 1024
F = 512            # HX * R
COLS = 16384       # W columns per core
NB = 32            # 512-col param blocks per core
NQ = 4             # collective stages
NBQ = NB // NQ     # nb blocks per stage
SPC = B // NCORES  # samples per core
TS = 512           # tokens per block
TB = N // TS

LAST_EXEC_NS = None
_cached_nc = None


def _ensure_axon_hooks():
    try:
        import antenv.axon_hooks  # noqa: F401
        return
    except Exception:
        pass
    hook = None
    try:
        import trn_agent_boot.trn_boot as tb
        hook = tb._ntff_profile_via_ctypes("/opt/axon/libaxon_pjrt.so")
    except Exception:
        hook = None
    mod = types.ModuleType("antenv.axon_hooks")
    mod.get_axon_ntff_profile_hook = lambda: hook
    sys.modules["antenv.axon_hooks"] = mod
    try:
        bass_utils.upload_artifacts = lambda tmpdir: tmpdir
    except Exception:
        pass


def _build():
    fp32 = mybir.dt.float32
    bf16 = mybir.dt.bfloat16
    AF = mybir.ActivationFunctionType
    ALU = mybir.AluOpType

    nc = bacc.Bacc("TRN2", target_bir_lowering=False, debug=False,
                   num_devices=NCORES)
    W_d = nc.dram_tensor("W", [NB, 128, 8 * F], bf16, kind="ExternalInput")
    x_d = nc.dram_tensor("x", [SPC, HX, N], bf16, kind="ExternalInput")
    s_d = nc.dram_tensor("s", [128, 8 * B], bf16, kind="ExternalInput")
    b_d = nc.dram_tensor("b", [B, NB * F], bf16, kind="ExternalInput")
    g_d = nc.dram_tensor("g", [HX, 1], fp32, kind="ExternalInput")
    o_d = nc.dram_tensor("o", [SPC, HX, N], bf16, kind="ExternalOutput")

    with tile.TileContext(nc) as tc:
        with tc.tile_pool(name="pers", bufs=1) as pers, \
             tc.tile_pool(name="xres", bufs=1) as xres, \
             tc.tile_pool(name="dram", bufs=1, space="DRAM") as dram:
            s_t = pers.tile([128, 8 * B], bf16)
            nc.sync.dma_start(s_t[:], s_d[:])
            g_t = pers.tile([HX, 1], fp32)
            nc.sync.dma_start(g_t[:], g_d[:])
            ones_col = pers.tile([128, 1], bf16)
            nc.vector.memset(ones_col[:], 1.0)
            ones_row = pers.tile([1, 128], bf16)
            nc.vector.memset(ones_row[:], 1.0)
            eps_t = pers.tile([128, 1], fp32)
            nc.vector.memset(eps_t[:], 1e-6)
            ident = pers.tile([128, 128], fp32)
            masks.make_identity(nc, ident[:])

            b_all = pers.tile([B, NB * F], bf16)
            nc.sync.dma_start(b_all[:], b_d[:])

            in_all = dram.tile([NQ, B, NBQ * F], bf16, name="in_all")
            out_all = dram.tile([NQ, B, NBQ * F], bf16, name="out_all")

            # phase A: params = s @ W + b for this core's 16384 columns, in
            # stages; each stage's all-to-all overlaps the next stage's
            # compute.
            with tc.tile_pool(name="wp", bufs=4) as wp, \
                 tc.tile_pool(name="stg", bufs=2) as stg, \
                 tc.tile_pool(name="psA", bufs=2, space="PSUM") as psA:
                for q in range(NQ):
                    for nbl in range(NBQ):
                        nb = q * NBQ + nbl
                        wt = wp.tile([128, 8 * F], bf16)
                        if nb == 0:
                            # split the first W load so the PE can start on
                            # the first k-chunks ~4us earlier
                            nc.sync.dma_start(wt[:, :2 * F],
                                              W_d[0, :, :2 * F])
                            nc.sync.dma_start(wt[:, 2 * F:],
                                              W_d[0, :, 2 * F:])
                        else:
                            nc.sync.dma_start(wt[:], W_d[nb, :, :])
                        ps = psA.tile([B, F], fp32)
                        for kt in range(8):
                            nc.tensor.matmul(
                                ps[:],
                                s_t[:, kt * B:(kt + 1) * B],
                                wt[:, kt * F:(kt + 1) * F],
                                start=(kt == 0), stop=(kt == 7),
                            )
                        st = stg.tile([B, F], bf16)
                        nc.vector.tensor_tensor(
                            st[:], ps[:],
                            b_all[:, nb * F:(nb + 1) * F], ALU.add)
                        nc.gpsimd.dma_start(
                            in_all[q, :, nbl * F:(nbl + 1) * F], st[:])
                    nc.gpsimd.collective_compute(
                        "AllToAll", ALU.bypass,
                        replica_groups=[list(range(NCORES))],
                        ins=[in_all[q].opt()], outs=[out_all[q].opt()],
                    )

            # x loads go on the SP DMA ring AFTER all W traffic so they do
            # not steal phase A bandwidth; they feed the rmsnorm stats that
            # run in the final collective's shadow.
            xts = []
            for i in range(SPC):
                xt = xres.tile([HX, N], bf16, name=f"xt{i}")
                nc.sync.dma_start(xt[:], x_d[i, :, :])
                xts.append(xt)

            # rmsnorm stats -> rr_flat[i] [1, N] bf16, entry t = 1/rms of
            # token t.
            rr_list = []
            with tc.tile_pool(name="xsqp", bufs=2) as xsqp, \
                 tc.tile_pool(name="stm", bufs=2) as stm, \
                 tc.tile_pool(name="psS", bufs=2, space="PSUM") as psS:
                for i in range(SPC):
                    xt = xts[i]
                    xsq = xsqp.tile([HX, N], bf16)
                    for ch in range(4):
                        sl = slice(ch * (N // 4), (ch + 1) * (N // 4))
                        nc.vector.tensor_tensor(xsq[:, sl], xt[:, sl],
                                                xt[:, sl], ALU.mult)
                    pn_s = psS.tile([128, 32], fp32, name="pn_s")
                    for c in range(32):
                        nc.tensor.matmul(
                            pn_s[:, c:c + 1],
                            xsq[:, c * 128:(c + 1) * 128],
                            ones_col[:],
                            start=True, stop=True,
                        )
                    sq_m = stm.tile([128, 32], fp32, name="sq_m")
                    nc.scalar.activation(sq_m[:], pn_s[:], AF.Sqrt,
                                         bias=eps_t[:], scale=1.0 / HX)
                    rr = stm.tile([128, 32], fp32, name="rr")
                    nc.vector.reciprocal(rr[:], sq_m[:])
                    rr_t = psS.tile([32, 128], fp32, name="rr_t")
                    nc.tensor.transpose(rr_t[:], rr[:], ident[:])
                    rr_ts = stm.tile([32, 128], bf16, name="rr_ts")
                    nc.vector.tensor_copy(rr_ts[:], rr_t[:])
                    # flatten token-major onto one partition so the block
                    # loop can broadcast [1, TS] rows with base partition 0
                    rr_flat = pers.tile([1, N], bf16, name=f"rr_flat{i}")
                    nc.sync.dma_start(rr_flat[:], rr_ts[:])
                    rr_list.append(rr_flat)

            # phase C: per-sample weight norms, then a software-pipelined
            # loop over (sample, token-block) units where bmm1 of unit u
            # overlaps bmm2 of unit u-1.
            with ExitStack() as es:
                def pool(name, bufs, space=None):
                    kw = {"space": space} if space else {}
                    return es.enter_context(
                        tc.tile_pool(name=name, bufs=bufs, **kw))
                p_fc1 = pool("fc1", 1)
                p_fc1g = pool("fc1g", 1)
                p_fc2 = pool("fc2", 1)
                p_sq = pool("sq", 1)
                p_rn = pool("rn", 1)
                p_tmp = pool("tmp", 2)
                p_xs = pool("xs", 2)
                p_h1 = pool("h1", 2)
                p_ob = pool("ob", 2)
                p_pn = pool("pn", 1, "PSUM")
                p_rnb = pool("rnb", 1, "PSUM")
                p_rrb = pool("rrb", 2)
                p_ph2 = pool("ph2", 2, "PSUM")
                p_ph1a = pool("ph1a", 1, "PSUM")
                p_ph1b = pool("ph1b", 1, "PSUM")

                fc1rs, fc2cs = [], []
                for i in range(SPC):
                    fc1r = p_fc1.tile([HX, F], bf16, name=f"fc1r{i}")
                    for src in range(4):
                        r = 4 * src + i
                        nc.sync.dma_start(
                            fc1r[32 * src:32 * (src + 1), :],
                            out_all[:, r:r + 1, :].rearrange(
                                "q o (a f) -> q (o a) f", a=32 // NQ),
                        )
                    fc2c = p_fc2.tile([128, 4 * HX], bf16, name=f"fc2c{i}")
                    for fb in range(4):
                        r = 16 + 4 * fb + i
                        nc.sync.dma_start(
                            fc2c[:, fb * HX:(fb + 1) * HX],
                            out_all[:, r:r + 1, :].rearrange(
                                "q o (p d) -> q (o p) d", p=128 // NQ),
                        )
                    fc1rs.append(fc1r)
                    fc2cs.append(fc2c)

                # squared weights for the column norms (Pool engine)
                sq1s, sq2s = [], []
                for i in range(SPC):
                    sq1 = p_sq.tile([HX, F], bf16, name=f"sq1_{i}")
                    nc.gpsimd.tensor_tensor(sq1[:], fc1rs[i][:], fc1rs[i][:],
                                            ALU.mult)
                    sq2 = p_sq.tile([128, F], bf16, name=f"sq2_{i}")
                    nc.gpsimd.tensor_tensor(sq2[:], fc2cs[i][:], fc2cs[i][:],
                                            ALU.mult)
                    sq1s.append(sq1)
                    sq2s.append(sq2)

                # all norms in one PSUM tile: cols (i, fb) = fc1 norms,
                # cols 16+i = fc2 norms
                pnall = p_pn.tile([128, 20], fp32, name="pscr")
                for i in range(SPC):
                    for fb in range(4):
                        nc.tensor.matmul(
                            pnall[:, 4 * i + fb:4 * i + fb + 1],
                            sq1s[i][:, fb * 128:(fb + 1) * 128],
                            ones_col[:], start=True, stop=True)
                for i in range(SPC):
                    for fb in range(4):
                        nc.tensor.matmul(
                            pnall[:, 16 + i:17 + i],
                            sq2s[i][:, fb * 128:(fb + 1) * 128],
                            ones_col[:], start=(fb == 0), stop=(fb == 3))
                n_all = p_tmp.tile([128, 20], fp32)
                nc.scalar.activation(n_all[:], pnall[:], AF.Sqrt)
                rn_all = p_rn.tile([128, 20], fp32, name="rn_all")
                nc.vector.reciprocal(rn_all[:], n_all[:])

                # fc1 norms to a flat [1, 16*128] row (token-major) so each
                # sample's [1, F] slice can broadcast over d in one matmul
                trn = p_pn.tile([16, 128], fp32, name="pscr")
                nc.tensor.transpose(trn[:], rn_all[:, 0:16], ident[:])
                rn1t = p_tmp.tile([16, 128], bf16)
                nc.vector.tensor_copy(rn1t[:], trn[:])
                rn1_flat = pers.tile([1, 16 * 128], bf16, name="rn1_flat")
                nc.sync.dma_start(rn1_flat[:], rn1t[:])

                fc1gs, rn2s = [], []
                for i in range(SPC):
                    rn1b = p_rnb.tile([HX, F], fp32, name="rn1b")
                    nc.tensor.matmul(rn1b[:], ones_row[:],
                                     rn1_flat[0:1, i * F:(i + 1) * F],
                                     start=True, stop=True)
                    fc1g = p_fc1g.tile([HX, F], bf16, name=f"fc1g{i}")
                    nc.vector.scalar_tensor_tensor(
                        fc1g[:], fc1rs[i][:], g_t[:], rn1b[:],
                        ALU.mult, ALU.mult)
                    fc1gs.append(fc1g)
                    rn2s.append(rn_all[:, 16 + i:17 + i])

                units = [(i, tb) for i in range(SPC) for tb in range(TB)]
                prev = None

                def bmm2_first(pv):
                    nc.tensor.matmul(pv["ph2"][:],
                                     fc2cs[pv["i"]][:, 0:HX],
                                     pv["h1a"][:, 0:TS],
                                     start=True, stop=False)
                    nc.tensor.matmul(pv["ph2"][:],
                                     fc2cs[pv["i"]][:, HX:2 * HX],
                                     pv["h1a"][:, TS:2 * TS],
                                     start=False, stop=False)

                def bmm2_second(pv):
                    nc.tensor.matmul(pv["ph2"][:],
                                     fc2cs[pv["i"]][:, 2 * HX:3 * HX],
                                     pv["h1b"][:, 0:TS],
                                     start=False, stop=False)
                    nc.tensor.matmul(pv["ph2"][:],
                                     fc2cs[pv["i"]][:, 3 * HX:4 * HX],
                                     pv["h1b"][:, TS:2 * TS],
                                     start=False, stop=True)
                    ob = p_ob.tile([HX, TS], bf16)
                    nc.vector.scalar_tensor_tensor(
                        ob[:], pv["ph2"][:], rn2s[pv["i"]][:], pv["xv"],
                        ALU.mult, ALU.add)
                    nc.sync.dma_start(
                        o_d[pv["i"], :, pv["tb"] * TS:(pv["tb"] + 1) * TS],
                        ob[:])

                for i, tb in units:
                    xt = xts[i]
                    xv = xt[:, tb * TS:(tb + 1) * TS]
                    rrb = p_rrb.tile([HX, TS], bf16, name="rrb")
                    nc.gpsimd.partition_broadcast(
                        rrb[:], rr_list[i][0:1, tb * TS:(tb + 1) * TS])
                    xs = p_xs.tile([HX, TS], bf16)
                    nc.vector.tensor_tensor(xs[:], xv, rrb[:], ALU.mult)

                    fc1g = fc1gs[i]
                    ph1a = p_ph1a.tile([128, 2 * TS], fp32)
                    nc.tensor.matmul(ph1a[:, 0:TS], fc1g[:, 0:128], xs[:],
                                     start=True, stop=True)
                    nc.tensor.matmul(ph1a[:, TS:2 * TS], fc1g[:, 128:256],
                                     xs[:], start=True, stop=True)
                    h1a = p_h1.tile([128, 2 * TS], bf16)
                    nc.scalar.activation(h1a[:], ph1a[:], AF.Silu)

                    if prev is not None:
                        prev["ph2"] = p_ph2.tile([HX, TS], fp32, name="ph2")
                        bmm2_first(prev)

                    ph1b = p_ph1b.tile([128, 2 * TS], fp32)
                    nc.tensor.matmul(ph1b[:, 0:TS], fc1g[:, 256:384], xs[:],
                                     start=True, stop=True)
                    nc.tensor.matmul(ph1b[:, TS:2 * TS], fc1g[:, 384:512],
                                     xs[:], start=True, stop=True)
                    h1b = p_h1.tile([128, 2 * TS], bf16)
                    nc.scalar.activation(h1b[:], ph1b[:], AF.Silu)

                    if prev is not None:
                        bmm2_second(prev)

                    prev = {"i": i, "tb": tb, "xv": xv,
                            "h1a": h1a, "h1b": h1b}

                prev["ph2"] = p_ph2.tile([HX, TS], fp32, name="ph2")
                bmm2_first(prev)
                bmm2_second(prev)
    nc.compile()
    return nc


def _prep_inputs(x, s, W, b, g):
    s_p = np.ascontiguousarray(
        s.T.reshape(8, 128, B).transpose(1, 0, 2).reshape(128, 8 * B)
    ).astype(np_bf16)
    g_p = np.ascontiguousarray(g.reshape(HX, 1)).astype(np.float32)
    Wb = W.astype(np_bf16)
    bb = b.astype(np_bf16)
    in_maps = []
    for c in range(NCORES):
        Wc = Wb[:, c * COLS:(c + 1) * COLS]
        Wc = np.ascontiguousarray(
            Wc.reshape(8, 128, NB, F).transpose(2, 1, 0, 3)
              .reshape(NB, 128, 8 * F))
        bc = np.ascontiguousarray(np.broadcast_to(
            bb[c * COLS:(c + 1) * COLS].reshape(NB, 1, F), (NB, B, F)))
        xc = np.ascontiguousarray(
            x[SPC * c:SPC * (c + 1)].transpose(0, 2, 1)).astype(np_bf16)
        in_maps.append({"W": Wc, "x": xc, "s": s_p, "b": bc, "g": g_p})
    return in_maps


def kernel(x, s, W, b, g):
    global LAST_EXEC_NS, _cached_nc
    x = np.asarray(x, dtype=np.float32)
    s = np.asarray(s, dtype=np.float32)
    W = np.asarray(W, dtype=np.float32)
    b = np.asarray(b, dtype=np.float32)
    g = np.asarray(g, dtype=np.float32)

    trace = os.environ.get("KERNEL_TRACE", "0") == "1"
    if trace:
        _ensure_axon_hooks()
    if _cached_nc is None:
        _cached_nc = _build()
    in_maps = _prep_inputs(x, s, W, b, g)
    res = run_bass_kernel_spmd(_cached_nc, in_maps, list(range(NCORES)),
                               trace=trace)
    LAST_EXEC_NS = res.exec_time_ns
    out = np.concatenate([res.results[c]["o"] for c in range(NCORES)], axis=0)
    return np.ascontiguousarray(
        out.transpose(0, 2, 1).astype(np.float32))


# revision 26
# speedup vs baseline: 1.0094x; 1.0094x over previous
import os
import sys
import types
from contextlib import ExitStack

sys.path.insert(0, "/opt/trn_rl_repo")

import numpy as np
from ml_dtypes import bfloat16 as np_bf16

import concourse.bacc as bacc
import concourse.tile as tile
import concourse.mybir as mybir
from concourse import bass_utils, masks
from concourse.bass_utils import run_bass_kernel_spmd

NCORES = 8
B, N, HX, HS = 32, 4096, 128, 1024
F = 512            # HX * R
COLS = 16384       # W columns per core
NB = 32            # 512-col param blocks per core
NQ = 4             # collective stages
NBQ = NB // NQ     # nb blocks per stage
SPC = B // NCORES  # samples per core
TS = 512           # tokens per block
TB = N // TS

LAST_EXEC_NS = None
_cached_nc = None


def _ensure_axon_hooks():
    try:
        import antenv.axon_hooks  # noqa: F401
        return
    except Exception:
        pass
    hook = None
    try:
        import trn_agent_boot.trn_boot as tb
        hook = tb._ntff_profile_via_ctypes("/opt/axon/libaxon_pjrt.so")
    except Exception:
        hook = None
    mod = types.ModuleType("antenv.axon_hooks")
    mod.get_axon_ntff_profile_hook = lambda: hook
    sys.modules["antenv.axon_hooks"] = mod
    try:
        bass_utils.upload_artifacts = lambda tmpdir: tmpdir
    except Exception:
        pass


def _build():
    fp32 = mybir.dt.float32
    bf16 = mybir.dt.bfloat16
    AF = mybir.ActivationFunctionType
    ALU = mybir.AluOpType

    nc = bacc.Bacc("TRN2", target_bir_lowering=False, debug=False,
                   num_devices=NCORES)
    W_d = nc.dram_tensor("W", [NB, 128, 8 * F], bf16, kind="ExternalInput")
    x_d = nc.dram_tensor("x", [SPC, HX, N], bf16, kind="ExternalInput")
    s_d = nc.dram_tensor("s", [128, 8 * B], bf16, kind="ExternalInput")
    b_d = nc.dram_tensor("b", [B, NB * F], bf16, kind="ExternalInput")
    g_d = nc.dram_tensor("g", [HX, 1], fp32, kind="ExternalInput")
    o_d = nc.dram_tensor("o", [SPC, HX, N], bf16, kind="ExternalOutput")

    with tile.TileContext(nc) as tc:
        with tc.tile_pool(name="pers", bufs=1) as pers, \
             tc.tile_pool(name="xres", bufs=1) as xres, \
             tc.tile_pool(name="dram", bufs=1, space="DRAM") as dram:
            s_t = pers.tile([128, 8 * B], bf16)
            nc.sync.dma_start(s_t[:], s_d[:])
            g_t = pers.tile([HX, 1], fp32)
            nc.sync.dma_start(g_t[:], g_d[:])
            ones_col = pers.tile([128, 1], bf16)
            nc.vector.memset(ones_col[:], 1.0)
            ones_row = pers.tile([1, 128], bf16)
            nc.vector.memset(ones_row[:], 1.0)
            eps_t = pers.tile([128, 1], fp32)
            nc.vector.memset(eps_t[:], 1e-6)
            ident = pers.tile([128, 128], fp32)
            masks.make_identity(nc, ident[:])

            b_all = pers.tile([B, NB * F], bf16)
            nc.sync.dma_start(b_all[:], b_d[:])

            in_all = dram.tile([NQ, B, NBQ * F], bf16, name="in_all")
            out_all = dram.tile([NQ, B, NBQ * F], bf16, name="out_all")

            # phase A: params = s @ W + b for this core's 16384 columns, in
            # stages; each stage's all-to-all overlaps the next stage's
            # compute.
            with tc.tile_pool(name="wp", bufs=4) as wp, \
                 tc.tile_pool(name="stg", bufs=2) as stg, \
                 tc.tile_pool(name="psA", bufs=2, space="PSUM") as psA:
                for q in range(NQ):
                    for nbl in range(NBQ):
                        nb = q * NBQ + nbl
                        wt = wp.tile([128, 8 * F], bf16)
                        if nb == 0:
                            # split the first W load so the PE can start on
                            # the first k-chunks ~4us earlier
                            nc.sync.dma_start(wt[:, :2 * F],
                                              W_d[0, :, :2 * F])
                            nc.sync.dma_start(wt[:, 2 * F:],
                                              W_d[0, :, 2 * F:])
                        else:
                            nc.sync.dma_start(wt[:], W_d[nb, :, :])
                        ps = psA.tile([B, F], fp32)
                        for kt in range(8):
                            nc.tensor.matmul(
                                ps[:],
                                s_t[:, kt * B:(kt + 1) * B],
                                wt[:, kt * F:(kt + 1) * F],
                                start=(kt == 0), stop=(kt == 7),
                            )
                        st = stg.tile([B, F], bf16)
                        nc.vector.tensor_tensor(
                            st[:], ps[:],
                            b_all[:, nb * F:(nb + 1) * F], ALU.add)
                        nc.gpsimd.dma_start(
                            in_all[q, :, nbl * F:(nbl + 1) * F], st[:])
                    nc.gpsimd.collective_compute(
                        "AllToAll", ALU.bypass,
                        replica_groups=[list(range(NCORES))],
                        ins=[in_all[q].opt()], outs=[out_all[q].opt()],
                    )

            # x loads go on the SP DMA ring AFTER all W traffic so they do
            # not steal phase A bandwidth; they feed the rmsnorm stats that
            # run in the final collective's shadow.
            xts = []
            for i in range(SPC):
                xt = xres.tile([HX, N], bf16, name=f"xt{i}")
                nc.sync.dma_start(xt[:], x_d[i, :, :])
                xts.append(xt)

            # rmsnorm stats -> rr_flat[i] [1, N] bf16, entry t = 1/rms of
            # token t.
            rr_list = []
            with tc.tile_pool(name="xsqp", bufs=2) as xsqp, \
                 tc.tile_pool(name="stm", bufs=2) as stm, \
                 tc.tile_pool(name="psS", bufs=2, space="PSUM") as psS:
                for i in range(SPC):
                    xt = xts[i]
                    xsq = xsqp.tile([HX, N], bf16)
                    for ch in range(4):
                        sl = slice(ch * (N // 4), (ch + 1) * (N // 4))
                        nc.vector.tensor_tensor(xsq[:, sl], xt[:, sl],
                                                xt[:, sl], ALU.mult)
                    pn_s = psS.tile([128, 32], fp32, name="pn_s")
                    for c in range(32):
                        nc.tensor.matmul(
                            pn_s[:, c:c + 1],
                            xsq[:, c * 128:(c + 1) * 128],
                            ones_col[:],
                            start=True, stop=True,
                        )
                    sq_m = stm.tile([128, 32], fp32, name="sq_m")
                    nc.scalar.activation(sq_m[:], pn_s[:], AF.Sqrt,
                                         bias=eps_t[:], scale=1.0 / HX)
                    rr = stm.tile([128, 32], fp32, name="rr")
                    nc.vector.reciprocal(rr[:], sq_m[:])
                    rr_t = psS.tile([32, 128], fp32, name="rr_t")
                    nc.tensor.transpose(rr_t[:], rr[:], ident[:])
                    rr_ts = stm.tile([32, 128], bf16, name="rr_ts")
                    nc.vector.tensor_copy(rr_ts[:], rr_t[:])
                    # flatten token-major onto one partition so the block
                    # loop can broadcast [1, TS] rows with base partition 0
                    rr_flat = pers.tile([1, N], bf16, name=f"rr_flat{i}")
                    nc.scalar.dma_start(rr_flat[:], rr_ts[:])
                    rr_list.append(rr_flat)

            # phase C: per-sample weight norms, then a software-pipelined
            # loop over (sample, token-block) units where bmm1 of unit u
            # overlaps bmm2 of unit u-1.
            with ExitStack() as es:
                def pool(name, bufs, space=None):
                    kw = {"space": space} if space else {}
                    return es.enter_context(
                        tc.tile_pool(name=name, bufs=bufs, **kw))
                p_fc1 = pool("fc1", 1)
                p_fc1g = pool("fc1g", 1)
                p_fc2 = pool("fc2", 1)
                p_sq = pool("sq", 1)
                p_rn = pool("rn", 1)
                p_tmp = pool("tmp", 2)
                p_xs = pool("xs", 2)
                p_h1 = pool("h1", 2)
                p_ob = pool("ob", 2)
                p_pn = pool("pn", 1, "PSUM")
                p_rnb = pool("rnb", 1, "PSUM")
                p_rrb = pool("rrb", 2)
                p_ph2 = pool("ph2", 2, "PSUM")
                p_ph1a = pool("ph1a", 1, "PSUM")
                p_ph1b = pool("ph1b", 1, "PSUM")

                fc1rs, fc2cs = [], []
                for i in range(SPC):
                    fc1r = p_fc1.tile([HX, F], bf16, name=f"fc1r{i}")
                    for src in range(4):
                        r = 4 * src + i
                        nc.sync.dma_start(
                            fc1r[32 * src:32 * (src + 1), :],
                            out_all[:, r:r + 1, :].rearrange(
                                "q o (a f) -> q (o a) f", a=32 // NQ),
                        )
                    fc2c = p_fc2.tile([128, 4 * HX], bf16, name=f"fc2c{i}")
                    for fb in range(4):
                        r = 16 + 4 * fb + i
                        nc.scalar.dma_start(
                            fc2c[:, fb * HX:(fb + 1) * HX],
                            out_all[:, r:r + 1, :].rearrange(
                                "q o (p d) -> q (o p) d", p=128 // NQ),
                        )
                    fc1rs.append(fc1r)
                    fc2cs.append(fc2c)

                # squared weights for the column norms (Pool engine)
                sq1s, sq2s = [], []
                for i in range(SPC):
                    sq1 = p_sq.tile([HX, F], bf16, name=f"sq1_{i}")
                    nc.gpsimd.tensor_tensor(sq1[:], fc1rs[i][:], fc1rs[i][:],
                                            ALU.mult)
                    sq2 = p_sq.tile([128, F], bf16, name=f"sq2_{i}")
                    nc.gpsimd.tensor_tensor(sq2[:], fc2cs[i][:], fc2cs[i][:],
                                            ALU.mult)
                    sq1s.append(sq1)
                    sq2s.append(sq2)

                # all norms in one PSUM tile: cols (i, fb) = fc1 norms,
                # cols 16+i = fc2 norms
                pnall = p_pn.tile([128, 20], fp32, name="pscr")
                for i in range(SPC):
                    for fb in range(4):
                        nc.tensor.matmul(
                            pnall[:, 4 * i + fb:4 * i + fb + 1],
                            sq1s[i][:, fb * 128:(fb + 1) * 128],
                            ones_col[:], start=True, stop=True)
                for i in range(SPC):
                    for fb in range(4):
                        nc.tensor.matmul(
                            pnall[:, 16 + i:17 + i],
                            sq2s[i][:, fb * 128:(fb + 1) * 128],
                            ones_col[:], start=(fb == 0), stop=(fb == 3))
                n_all = p_tmp.tile([128, 20], fp32)
                nc.scalar.activation(n_all[:], pnall[:], AF.Sqrt)
                rn_all = p_rn.tile([128, 20], fp32, name="rn_all")
                nc.vector.reciprocal(rn_all[:], n_all[:])

                # fc1 norms to a flat [1, 16*128] row (token-major) so each
                # sample's [1, F] slice can broadcast over d in one matmul
                trn = p_pn.tile([16, 128], fp32, name="pscr")
                nc.tensor.transpose(trn[:], rn_all[:, 0:16], ident[:])
                rn1t = p_tmp.tile([16, 128], bf16)
                nc.vector.tensor_copy(rn1t[:], trn[:])
                rn1_flat = pers.tile([1, 16 * 128], bf16, name="rn1_flat")
                nc.scalar.dma_start(rn1_flat[:], rn1t[:])

                fc1gs, rn2s = [], []
                for i in range(SPC):
                    rn1b = p_rnb.tile([HX, F], fp32, name="rn1b")
                    nc.tensor.matmul(rn1b[:], ones_row[:],
                                     rn1_flat[0:1, i * F:(i + 1) * F],
                                     start=True, stop=True)
                    fc1g = p_fc1g.tile([HX, F], bf16, name=f"fc1g{i}")
                    nc.vector.scalar_tensor_tensor(
                        fc1g[:], fc1rs[i][:], g_t[:], rn1b[:],
                        ALU.mult, ALU.mult)
                    fc1gs.append(fc1g)
                    rn2s.append(rn_all[:, 16 + i:17 + i])

                units = [(i, tb) for i in range(SPC) for tb in range(TB)]
                prev = None

                def bmm2_first(pv):
                    nc.tensor.matmul(pv["ph2"][:],
                                     fc2cs[pv["i"]][:, 0:HX],
                                     pv["h1a"][:, 0:TS],
                                     start=True, stop=False)
                    nc.tensor.matmul(pv["ph2"][:],
                                     fc2cs[pv["i"]][:, HX:2 * HX],
                                     pv["h1a"][:, TS:2 * TS],
                                     start=False, stop=False)

                def bmm2_second(pv):
                    nc.tensor.matmul(pv["ph2"][:],
                                     fc2cs[pv["i"]][:, 2 * HX:3 * HX],
                                     pv["h1b"][:, 0:TS],
                                     start=False, stop=False)
                    nc.tensor.matmul(pv["ph2"][:],
                                     fc2cs[pv["i"]][:, 3 * HX:4 * HX],
                                     pv["h1b"][:, TS:2 * TS],
                                     start=False, stop=True)
                    ob = p_ob.tile([HX, TS], bf16)
                    nc.vector.scalar_tensor_tensor(
                        ob[:], pv["ph2"][:], rn2s[pv["i"]][:], pv["xv"],
                        ALU.mult, ALU.add)
                    nc.sync.dma_start(
                        o_d[pv["i"], :, pv["tb"] * TS:(pv["tb"] + 1) * TS],
                        ob[:])

                for i, tb in units:
                    xt = xts[i]
                    xv = xt[:, tb * TS:(tb + 1) * TS]
                    rrb = p_rrb.tile([HX, TS], bf16, name="rrb")
                    nc.gpsimd.partition_broadcast(
                        rrb[:], rr_list[i][0:1, tb * TS:(tb + 1) * TS])
                    xs = p_xs.tile([HX, TS], bf16)
                    nc.vector.tensor_tensor(xs[:], xv, rrb[:], ALU.mult)

                    fc1g = fc1gs[i]
                    ph1a = p_ph1a.tile([128, 2 * TS], fp32)
                    nc.tensor.matmul(ph1a[:, 0:TS], fc1g[:, 0:128], xs[:],
                                     start=True, stop=True)
                    nc.tensor.matmul(ph1a[:, TS:2 * TS], fc1g[:, 128:256],
                                     xs[:], start=True, stop=True)
                    h1a = p_h1.tile([128, 2 * TS], bf16)
                    nc.scalar.activation(h1a[:], ph1a[:], AF.Silu)

                    if prev is not None:
                        prev["ph2"] = p_ph2.tile([HX, TS], fp32, name="ph2")
                        bmm2_first(prev)

                    ph1b = p_ph1b.tile([128, 2 * TS], fp32)
                    nc.tensor.matmul(ph1b[:, 0:TS], fc1g[:, 256:384], xs[:],
                                     start=True, stop=True)
                    nc.tensor.matmul(ph1b[:, TS:2 * TS], fc1g[:, 384:512],
                                     xs[:], start=True, stop=True)
                    h1b = p_h1.tile([128, 2 * TS], bf16)
                    nc.scalar.activation(h1b[:], ph1b[:], AF.Silu)

                    if prev is not None:
                        bmm2_second(prev)

                    prev = {"i": i, "tb": tb, "xv": xv,
                            "h1a": h1a, "h1b": h1b}

                prev["ph2"] = p_ph2.tile([HX, TS], fp32, name="ph2")
                bmm2_first(prev)
                bmm2_second(prev)
    nc.compile()
    return nc


def _prep_inputs(x, s, W, b, g):
    s_p = np.ascontiguousarray(
        s.T.reshape(8, 128, B).transpose(1, 0, 2).reshape(128, 8 * B)
    ).astype(np_bf16)
    g_p = np.ascontiguousarray(g.reshape(HX, 1)).astype(np.float32)
    Wb = W.astype(np_bf16)
    bb = b.astype(np_bf16)
    in_maps = []
    for c in range(NCORES):
        Wc = Wb[:, c * COLS:(c + 1) * COLS]
        Wc = np.ascontiguousarray(
            Wc.reshape(8, 128, NB, F).transpose(2, 1, 0, 3)
              .reshape(NB, 128, 8 * F))
        bc = np.ascontiguousarray(np.broadcast_to(
            bb[c * COLS:(c + 1) * COLS].reshape(NB, 1, F), (NB, B, F)))
        xc = np.ascontiguousarray(
            x[SPC * c:SPC * (c + 1)].transpose(0, 2, 1)).astype(np_bf16)
        in_maps.append({"W": Wc, "x": xc, "s": s_p, "b": bc, "g": g_p})
    return in_maps


def kernel(x, s, W, b, g):
    global LAST_EXEC_NS, _cached_nc
    x = np.asarray(x, dtype=np.float32)
    s = np.asarray(s, dtype=np.float32)
    W = np.asarray(W, dtype=np.float32)
    b = np.asarray(b, dtype=np.float32)
    g = np.asarray(g, dtype=np.float32)

    trace = os.environ.get("KERNEL_TRACE", "0") == "1"
    if trace:
        _ensure_axon_hooks()
    if _cached_nc is None:
        _cached_nc = _build()
    in_maps = _prep_inputs(x, s, W, b, g)
    res = run_bass_kernel_spmd(_cached_nc, in_maps, list(range(NCORES)),
                               trace=trace)
    LAST_EXEC_NS = res.exec_time_ns
    out = np.concatenate([res.results[c]["o"] for c in range(NCORES)], axis=0)
    return np.ascontiguousarray(
        out.transpose(0, 2, 1).astype(np.float32))


# revision 27
# speedup vs baseline: 1.0536x; 1.0437x over previous
import os
import sys
import types
from contextlib import ExitStack

sys.path.insert(0, "/opt/trn_rl_repo")

import numpy as np
from ml_dtypes import bfloat16 as np_bf16

import concourse.bacc as bacc
import concourse.tile as tile
import concourse.mybir as mybir
from concourse import bass_utils, masks
from concourse.bass_utils import run_bass_kernel_spmd

NCORES = 8
B, N, HX, HS = 32, 4096, 128, 1024
F = 512            # HX * R
COLS = 16384       # W columns per core
NB = 32            # 512-col param blocks per core
NQ = 4             # collective stages
NBQ = NB // NQ     # nb blocks per stage
SPC = B // NCORES  # samples per core
TS = 512           # tokens per block
TB = N // TS

LAST_EXEC_NS = None
_cached_nc = None


def _ensure_axon_hooks():
    try:
        import antenv.axon_hooks  # noqa: F401
        return
    except Exception:
        pass
    hook = None
    try:
        import trn_agent_boot.trn_boot as tb
        hook = tb._ntff_profile_via_ctypes("/opt/axon/libaxon_pjrt.so")
    except Exception:
        hook = None
    mod = types.ModuleType("antenv.axon_hooks")
    mod.get_axon_ntff_profile_hook = lambda: hook
    sys.modules["antenv.axon_hooks"] = mod
    try:
        bass_utils.upload_artifacts = lambda tmpdir: tmpdir
    except Exception:
        pass


def _build():
    fp32 = mybir.dt.float32
    bf16 = mybir.dt.bfloat16
    AF = mybir.ActivationFunctionType
    ALU = mybir.AluOpType

    nc = bacc.Bacc("TRN2", target_bir_lowering=False, debug=False,
                   num_devices=NCORES)
    W_d = nc.dram_tensor("W", [NB, 128, 8 * F], bf16, kind="ExternalInput")
    x_d = nc.dram_tensor("x", [SPC, HX, N], bf16, kind="ExternalInput")
    s_d = nc.dram_tensor("s", [128, 8 * B], bf16, kind="ExternalInput")
    b_d = nc.dram_tensor("b", [B, NB * F], bf16, kind="ExternalInput")
    g_d = nc.dram_tensor("g", [HX, 1], fp32, kind="ExternalInput")
    o_d = nc.dram_tensor("o", [SPC, HX, N], bf16, kind="ExternalOutput")

    with tile.TileContext(nc) as tc:
        with tc.tile_pool(name="pers", bufs=1) as pers, \
             tc.tile_pool(name="xres", bufs=1) as xres, \
             tc.tile_pool(name="dram", bufs=1, space="DRAM") as dram:
            s_t = pers.tile([128, 8 * B], bf16)
            nc.sync.dma_start(s_t[:], s_d[:])
            g_t = pers.tile([HX, 1], fp32)
            nc.sync.dma_start(g_t[:], g_d[:])
            ones_col = pers.tile([128, 1], bf16)
            nc.vector.memset(ones_col[:], 1.0)
            ones_row = pers.tile([1, 128], bf16)
            nc.vector.memset(ones_row[:], 1.0)
            eps_t = pers.tile([128, 1], fp32)
            nc.vector.memset(eps_t[:], 1e-6)
            ident = pers.tile([128, 128], fp32)
            masks.make_identity(nc, ident[:])

            b_all = pers.tile([B, NB * F], bf16)
            nc.sync.dma_start(b_all[:], b_d[:])

            in_all = dram.tile([NQ, B, NBQ * F], bf16, name="in_all")
            out_all = dram.tile([NQ, B, NBQ * F], bf16, name="out_all")

            # phase A: params = s @ W + b for this core's 16384 columns, in
            # stages; each stage's all-to-all overlaps the next stage's
            # compute.
            with tc.tile_pool(name="wp", bufs=4) as wp, \
                 tc.tile_pool(name="stg", bufs=2) as stg, \
                 tc.tile_pool(name="psA", bufs=2, space="PSUM") as psA:
                for q in range(NQ):
                    for nbl in range(NBQ):
                        nb = q * NBQ + nbl
                        wt = wp.tile([128, 8 * F], bf16)
                        if nb == 0:
                            # split the first W load so the PE can start on
                            # the first k-chunks ~4us earlier
                            nc.sync.dma_start(wt[:, :2 * F],
                                              W_d[0, :, :2 * F])
                            nc.sync.dma_start(wt[:, 2 * F:],
                                              W_d[0, :, 2 * F:])
                        else:
                            nc.sync.dma_start(wt[:], W_d[nb, :, :])
                        ps = psA.tile([B, F], fp32)
                        for kt in range(8):
                            nc.tensor.matmul(
                                ps[:],
                                s_t[:, kt * B:(kt + 1) * B],
                                wt[:, kt * F:(kt + 1) * F],
                                start=(kt == 0), stop=(kt == 7),
                            )
                        st = stg.tile([B, F], bf16)
                        nc.vector.tensor_tensor(
                            st[:], ps[:],
                            b_all[:, nb * F:(nb + 1) * F], ALU.add)
                        nc.gpsimd.dma_start(
                            in_all[q, :, nbl * F:(nbl + 1) * F], st[:])
                    nc.gpsimd.collective_compute(
                        "AllToAll", ALU.bypass,
                        replica_groups=[list(range(NCORES))],
                        ins=[in_all[q].opt()], outs=[out_all[q].opt()],
                    )

            # x loads go on the SP DMA ring AFTER all W traffic so they do
            # not steal phase A bandwidth; they feed the rmsnorm stats that
            # run in the final collective's shadow.
            xts = []
            for i in range(SPC):
                xt = xres.tile([HX, N], bf16, name=f"xt{i}")
                nc.sync.dma_start(xt[:], x_d[i, :, :])
                xts.append(xt)

            # rmsnorm stats -> rr_flat[i] [1, N] bf16, entry t = 1/rms of
            # token t.
            rr_list = []
            with tc.tile_pool(name="xsqp", bufs=2) as xsqp, \
                 tc.tile_pool(name="stm", bufs=2) as stm, \
                 tc.tile_pool(name="psS", bufs=2, space="PSUM") as psS:
                for i in range(SPC):
                    xt = xts[i]
                    xsq = xsqp.tile([HX, N], bf16)
                    for ch in range(4):
                        sl = slice(ch * (N // 4), (ch + 1) * (N // 4))
                        nc.vector.tensor_tensor(xsq[:, sl], xt[:, sl],
                                                xt[:, sl], ALU.mult)
                    pn_s = psS.tile([128, 32], fp32, name="pn_s")
                    for c in range(32):
                        nc.tensor.matmul(
                            pn_s[:, c:c + 1],
                            xsq[:, c * 128:(c + 1) * 128],
                            ones_col[:],
                            start=True, stop=True,
                        )
                    sq_m = stm.tile([128, 32], fp32, name="sq_m")
                    nc.scalar.activation(sq_m[:], pn_s[:], AF.Sqrt,
                                         bias=eps_t[:], scale=1.0 / HX)
                    rr = stm.tile([128, 32], fp32, name="rr")
                    nc.vector.reciprocal(rr[:], sq_m[:])
                    rr_t = psS.tile([32, 128], fp32, name="rr_t")
                    nc.tensor.transpose(rr_t[:], rr[:], ident[:])
                    rr_ts = stm.tile([32, 128], bf16, name="rr_ts")
                    nc.vector.tensor_copy(rr_ts[:], rr_t[:])
                    # flatten token-major onto one partition so the block
                    # loop can broadcast [1, TS] rows with base partition 0
                    rr_flat = pers.tile([1, N], bf16, name=f"rr_flat{i}")
                    nc.scalar.dma_start(rr_flat[:], rr_ts[:])
                    rr_list.append(rr_flat)

            # phase C: per-sample weight norms, then a software-pipelined
            # loop over (sample, token-block) units where bmm1 of unit u
            # overlaps bmm2 of unit u-1.
            with ExitStack() as es:
                def pool(name, bufs, space=None):
                    kw = {"space": space} if space else {}
                    return es.enter_context(
                        tc.tile_pool(name=name, bufs=bufs, **kw))
                p_fc1 = pool("fc1", 1)
                p_fc1g = pool("fc1g", 1)
                p_fc2 = pool("fc2", 1)
                p_sq = pool("sq", 1)
                p_rn = pool("rn", 1)
                p_tmp = pool("tmp", 2)
                p_xs = pool("xs", 2)
                p_h1 = pool("h1", 2)
                p_ob = pool("ob", 2)
                p_pn = pool("pn", 1, "PSUM")
                p_rnb = pool("rnb", 1, "PSUM")
                p_rrb = pool("rrb", 2)
                p_ph2 = pool("ph2", 2, "PSUM")
                p_ph1a = pool("ph1a", 1, "PSUM")
                p_ph1b = pool("ph1b", 1, "PSUM")

                fc1rs, fc2cs = [], []
                for i in range(SPC):
                    fc1r = p_fc1.tile([HX, F], bf16, name=f"fc1r{i}")
                    for src in range(4):
                        r = 4 * src + i
                        nc.sync.dma_start(
                            fc1r[32 * src:32 * (src + 1), :],
                            out_all[:, r:r + 1, :].rearrange(
                                "q o (a f) -> q (o a) f", a=32 // NQ),
                        )
                    fc2c = p_fc2.tile([128, 4 * HX], bf16, name=f"fc2c{i}")
                    for fb in range(4):
                        r = 16 + 4 * fb + i
                        nc.scalar.dma_start(
                            fc2c[:, fb * HX:(fb + 1) * HX],
                            out_all[:, r:r + 1, :].rearrange(
                                "q o (p d) -> q (o p) d", p=128 // NQ),
                        )
                    fc1rs.append(fc1r)
                    fc2cs.append(fc2c)

                # squared weights for the column norms (Pool engine)
                sq1s, sq2s = [], []
                for i in range(SPC):
                    sq1 = p_sq.tile([HX, F], bf16, name=f"sq1_{i}")
                    nc.vector.tensor_tensor(sq1[:], fc1rs[i][:], fc1rs[i][:],
                                            ALU.mult)
                    sq2 = p_sq.tile([128, F], bf16, name=f"sq2_{i}")
                    nc.vector.tensor_tensor(sq2[:], fc2cs[i][:], fc2cs[i][:],
                                            ALU.mult)
                    sq1s.append(sq1)
                    sq2s.append(sq2)

                # all norms in one PSUM tile: cols (i, fb) = fc1 norms,
                # cols 16+i = fc2 norms
                pnall = p_pn.tile([128, 20], fp32, name="pscr")
                for i in range(SPC):
                    for fb in range(4):
                        nc.tensor.matmul(
                            pnall[:, 4 * i + fb:4 * i + fb + 1],
                            sq1s[i][:, fb * 128:(fb + 1) * 128],
                            ones_col[:], start=True, stop=True)
                for i in range(SPC):
                    for fb in range(4):
                        nc.tensor.matmul(
                            pnall[:, 16 + i:17 + i],
                            sq2s[i][:, fb * 128:(fb + 1) * 128],
                            ones_col[:], start=(fb == 0), stop=(fb == 3))
                n_all = p_tmp.tile([128, 20], fp32)
                nc.scalar.activation(n_all[:], pnall[:], AF.Sqrt)
                rn_all = p_rn.tile([128, 20], fp32, name="rn_all")
                nc.vector.reciprocal(rn_all[:], n_all[:])

                # fc1 norms to a flat [1, 16*128] row (token-major) so each
                # sample's [1, F] slice can broadcast over d in one matmul
                trn = p_pn.tile([16, 128], fp32, name="pscr")
                nc.tensor.transpose(trn[:], rn_all[:, 0:16], ident[:])
                rn1t = p_tmp.tile([16, 128], bf16)
                nc.vector.tensor_copy(rn1t[:], trn[:])
                rn1_flat = pers.tile([1, 16 * 128], bf16, name="rn1_flat")
                nc.scalar.dma_start(rn1_flat[:], rn1t[:])

                fc1gs, rn2s = [], []
                for i in range(SPC):
                    rn1b = p_rnb.tile([HX, F], fp32, name="rn1b")
                    nc.tensor.matmul(rn1b[:], ones_row[:],
                                     rn1_flat[0:1, i * F:(i + 1) * F],
                                     start=True, stop=True)
                    fc1g = p_fc1g.tile([HX, F], bf16, name=f"fc1g{i}")
                    nc.vector.scalar_tensor_tensor(
                        fc1g[:], fc1rs[i][:], g_t[:], rn1b[:],
                        ALU.mult, ALU.mult)
                    fc1gs.append(fc1g)
                    rn2s.append(rn_all[:, 16 + i:17 + i])

                units = [(i, tb) for i in range(SPC) for tb in range(TB)]
                prev = None

                def bmm2_first(pv):
                    nc.tensor.matmul(pv["ph2"][:],
                                     fc2cs[pv["i"]][:, 0:HX],
                                     pv["h1a"][:, 0:TS],
                                     start=True, stop=False)
                    nc.tensor.matmul(pv["ph2"][:],
                                     fc2cs[pv["i"]][:, HX:2 * HX],
                                     pv["h1a"][:, TS:2 * TS],
                                     start=False, stop=False)

                def bmm2_second(pv):
                    nc.tensor.matmul(pv["ph2"][:],
                                     fc2cs[pv["i"]][:, 2 * HX:3 * HX],
                                     pv["h1b"][:, 0:TS],
                                     start=False, stop=False)
                    nc.tensor.matmul(pv["ph2"][:],
                                     fc2cs[pv["i"]][:, 3 * HX:4 * HX],
                                     pv["h1b"][:, TS:2 * TS],
                                     start=False, stop=True)
                    ob = p_ob.tile([HX, TS], bf16)
                    nc.vector.scalar_tensor_tensor(
                        ob[:], pv["ph2"][:], rn2s[pv["i"]][:], pv["xv"],
                        ALU.mult, ALU.add)
                    nc.sync.dma_start(
                        o_d[pv["i"], :, pv["tb"] * TS:(pv["tb"] + 1) * TS],
                        ob[:])

                for i, tb in units:
                    xt = xts[i]
                    xv = xt[:, tb * TS:(tb + 1) * TS]
                    rrb = p_rrb.tile([HX, TS], bf16, name="rrb")
                    nc.gpsimd.partition_broadcast(
                        rrb[:], rr_list[i][0:1, tb * TS:(tb + 1) * TS])
                    xs = p_xs.tile([HX, TS], bf16)
                    nc.vector.tensor_tensor(xs[:], xv, rrb[:], ALU.mult)

                    fc1g = fc1gs[i]
                    ph1a = p_ph1a.tile([128, 2 * TS], fp32)
                    nc.tensor.matmul(ph1a[:, 0:TS], fc1g[:, 0:128], xs[:],
                                     start=True, stop=True)
                    nc.tensor.matmul(ph1a[:, TS:2 * TS], fc1g[:, 128:256],
                                     xs[:], start=True, stop=True)
                    h1a = p_h1.tile([128, 2 * TS], bf16)
                    nc.scalar.activation(h1a[:], ph1a[:], AF.Silu)

                    if prev is not None:
                        prev["ph2"] = p_ph2.tile([HX, TS], fp32, name="ph2")
                        bmm2_first(prev)

                    ph1b = p_ph1b.tile([128, 2 * TS], fp32)
                    nc.tensor.matmul(ph1b[:, 0:TS], fc1g[:, 256:384], xs[:],
                                     start=True, stop=True)
                    nc.tensor.matmul(ph1b[:, TS:2 * TS], fc1g[:, 384:512],
                                     xs[:], start=True, stop=True)
                    h1b = p_h1.tile([128, 2 * TS], bf16)
                    nc.scalar.activation(h1b[:], ph1b[:], AF.Silu)

                    if prev is not None:
                        bmm2_second(prev)

                    prev = {"i": i, "tb": tb, "xv": xv,
                            "h1a": h1a, "h1b": h1b}

                prev["ph2"] = p_ph2.tile([HX, TS], fp32, name="ph2")
                bmm2_first(prev)
                bmm2_second(prev)
    nc.compile()
    return nc


def _prep_inputs(x, s, W, b, g):
    s_p = np.ascontiguousarray(
        s.T.reshape(8, 128, B).transpose(1, 0, 2).reshape(128, 8 * B)
    ).astype(np_bf16)
    g_p = np.ascontiguousarray(g.reshape(HX, 1)).astype(np.float32)
    Wb = W.astype(np_bf16)
    bb = b.astype(np_bf16)
    in_maps = []
    for c in range(NCORES):
        Wc = Wb[:, c * COLS:(c + 1) * COLS]
        Wc = np.ascontiguousarray(
            Wc.reshape(8, 128, NB, F).transpose(2, 1, 0, 3)
              .reshape(NB, 128, 8 * F))
        bc = np.ascontiguousarray(np.broadcast_to(
            bb[c * COLS:(c + 1) * COLS].reshape(NB, 1, F), (NB, B, F)))
        xc = np.ascontiguousarray(
            x[SPC * c:SPC * (c + 1)].transpose(0, 2, 1)).astype(np_bf16)
        in_maps.append({"W": Wc, "x": xc, "s": s_p, "b": bc, "g": g_p})
    return in_maps


def kernel(x, s, W, b, g):
    global LAST_EXEC_NS, _cached_nc
    x = np.asarray(x, dtype=np.float32)
    s = np.asarray(s, dtype=np.float32)
    W = np.asarray(W, dtype=np.float32)
    b = np.asarray(b, dtype=np.float32)
    g = np.asarray(g, dtype=np.float32)

    trace = os.environ.get("KERNEL_TRACE", "0") == "1"
    if trace:
        _ensure_axon_hooks()
    if _cached_nc is None:
        _cached_nc = _build()
    in_maps = _prep_inputs(x, s, W, b, g)
    res = run_bass_kernel_spmd(_cached_nc, in_maps, list(range(NCORES)),
                               trace=trace)
    LAST_EXEC_NS = res.exec_time_ns
    out = np.concatenate([res.results[c]["o"] for c in range(NCORES)], axis=0)
    return np.ascontiguousarray(
        out.transpose(0, 2, 1).astype(np.float32))
